# revision 20
# baseline (speedup 1.0000x reference)
"""Trainium2 Bass kernel for CausalConv1dUpdate (depthwise causal conv,
width=4, with state gather/scatter and SiLU).

Full-input contract: kernel(**inputs) takes the complete tensors, shards
the batch across 8 NeuronCores (data-parallel), runs the Bass kernel via
run_bass_kernel_spmd, and reassembles full outputs on the host.

Host prep: gathers conv_state rows by conv_state_indices and concatenates
[state, x] along time (one contiguous DMA source per core; the TT ISA
struct only carries one sync wait, so one input DMA per block is needed),
and pre-broadcasts weight+bias into the partition layout.

Device layout (per core, 32 sequences):
  - Two compute blocks of 16 sequences each.
  - SBUF partition dim = (c, j): c = d-chunk (8 of 512), j = seq-in-block
    (16) -> 128 partitions (p = c*16 + j); free dim = (t, u): t = time
    slot (19 = 3 state + 16 x), u = d-within-chunk. c-major packing keeps
    every DMA access pattern mergeable to <=3 dims.
  - conv: acc[t] = bias + sum_k w[k] * x_new[t+k]  (fp32, matching the
    reference accumulation order), then SiLU on the scalar engine.
  - new_state = x_new[:, -3:, :] = x[:, 13:16, :] is a pure slice of the
    input, so the host assembles updated_conv_state directly from x
    (no device traffic for it).
"""

import numpy as np

BATCH, SEQ, DIM, WIDTH, POOL = 256, 16, 4096, 4, 512
NCORES = 8
BPC = BATCH // NCORES       # sequences per core: 32
JBLK = 16                   # sequences per compute block
NBLK = BPC // JBLK          # 2
CCH = 8                     # d-chunks per partition group
U = DIM // CCH              # 512
T_XN = WIDTH - 1 + SEQ      # 19 time slots in x_new
ST = WIDTH - 1              # 3 state slots

_BUILT = None
# Test-harness knobs (ignored in normal use): set TRACE=True before calling
# kernel() to profile the run; the BassKernelResults lands in LAST_RESULTS.
TRACE = False
LAST_RESULTS = None


def _make_tc_class():
    """TileContext whose tail drain carries at most one sync wait per
    instruction — this walrus build's codegen rejects multi-wait sync
    info on every ISA struct, including CTRL (Drain)."""
    import concourse.mybir as mybir
    from bass_rust import ScopedClock
    from concourse.tile import TileContext

    class SplitDrainTileContext(TileContext):
        def _drain_and_barrier(self, tick_clock, wait_clock):
            drain_inst = self.nc.sync.drain()
            wait_clock.add_sem_waits(
                drain_inst.ins, ScopedClock({None: tick_clock.global_clock})
            )
            si = drain_inst.ins.sync_info
            if si is not None and len(si.on_wait) > 1:
                waits = list(si.on_wait)
                drain_inst.ins.sync_info = mybir.SyncInfo(
                    on_wait=[waits[0]], on_update=list(si.on_update)
                )
                for w in waits[1:]:
                    d = self.nc.sync.drain()
                    d.ins.sync_info = mybir.SyncInfo(on_wait=[w], on_update=[])
            self.nc.all_engine_barrier()
            assert self.sems is not None
            popped = self.nc._tile_sem_poison_stack.pop()
            assert popped is self._sem_poison
            self.nc.clear_and_free_semaphores(list(self.sems.allocated().values()))
            self.nc.all_engine_barrier()

    return SplitDrainTileContext


def _build(act_name="Silu"):
    import concourse.bass as bass
    import concourse.mybir as mybir

    TileContext = _make_tc_class()

    f32 = mybir.dt.float32
    MUL = mybir.AluOpType.mult
    ADD = mybir.AluOpType.add
    SILU = getattr(mybir.ActivationFunctionType, act_name)

    # target_bir_lowering routes compilation through the full neuronx-cc
    # pipeline, which legalizes multi-wait sync info (the direct walrus
    # codegen path rejects >1 sync wait per instruction).
    nc = bass.Bass(target_bir_lowering=True)
    xs_s = nc.dram_tensor("xs_s", [BPC, T_XN, DIM], f32, kind="ExternalInput")
    # wb: weights+bias pre-broadcast on host to the partition layout:
    # wb[p, k, :] = weight[k, (p//JBLK)*U:(p//JBLK+1)*U] for k<WIDTH,
    # wb[p, WIDTH, :] = bias[(p//JBLK)*U:(p//JBLK+1)*U]
    wb = nc.dram_tensor("wb", [128, WIDTH + 1, U], f32, kind="ExternalInput")
    out_s = nc.dram_tensor("out_s", [BPC, SEQ, DIM], f32, kind="ExternalOutput")

    with TileContext(nc) as tc:
        with tc.tile_pool(name="wbp", bufs=1) as wbp, \
             tc.tile_pool(name="xnp", bufs=2) as xnp, \
             tc.tile_pool(name="accp", bufs=2) as accp, \
             tc.tile_pool(name="tmpp", bufs=1) as tmpp:
            wbt = wbp.tile([128, WIDTH + 1, U], f32)
            nc.sync.dma_start(out=wbt[:], in_=wb[:])
            # 1-element copy so DVE observes the wb-DMA semaphore here;
            # the conv TTs can only carry a single sync wait each.
            warm = wbp.tile([128, 1], f32, tag="warm")
            nc.vector.tensor_copy(warm[:], wbt[:, 0, 0:1])

            for b in range(NBLK):
                lo, hi = b * JBLK, (b + 1) * JBLK
                xn = xnp.tile([128, T_XN, U], f32, tag="xn")
                acc = accp.tile([128, SEQ, U], f32, tag="acc")

                xsrc = xs_s[lo:hi].rearrange("j t (c u) -> c (j t) u", u=U)
                nc.sync.dma_start(out=xn[:], in_=xsrc)

                # tap 0 into acc, then + bias, then taps 1..3 via tmp
                w0 = wbt[:, 0:1, :].broadcast_to([128, SEQ, U])
                nc.vector.tensor_tensor(acc[:], xn[:, 0:SEQ, :], w0, MUL)
                bias_b = wbt[:, WIDTH:WIDTH + 1, :].broadcast_to([128, SEQ, U])
                nc.vector.tensor_tensor(acc[:], acc[:], bias_b, ADD)
                for k in range(1, WIDTH):
                    tmp = tmpp.tile([128, SEQ, U], f32, tag="tmp")
                    wk = wbt[:, k:k + 1, :].broadcast_to([128, SEQ, U])
                    nc.vector.tensor_tensor(tmp[:], xn[:, k:k + SEQ, :], wk, MUL)
                    nc.vector.tensor_tensor(acc[:], acc[:], tmp[:], ADD)

                accf = acc[:].rearrange("p t u -> p (t u)")
                nc.scalar.activation(accf, accf, SILU)

                odst = out_s[lo:hi].rearrange("j t (c u) -> c (j t) u", u=U)
                # issue from the scalar engine: program order after the
                # in-place SiLU makes the RAW dep free (DMA ISA structs
                # carry only one sync wait)
                nc.scalar.dma_start(out=odst, in_=acc[:])

    return nc


def get_nc(act_name="Silu"):
    global _BUILT
    if _BUILT is None:
        _BUILT = _build(act_name)
    return _BUILT


def _prep_wb(weight, bias):
    wb = np.empty((128, WIDTH + 1, U), np.float32)
    c = np.arange(128) // JBLK
    wr = weight.reshape(WIDTH, CCH, U)
    for k in range(WIDTH):
        wb[:, k, :] = wr[k][c]
    wb[:, WIDTH, :] = bias.reshape(CCH, U)[c]
    return wb


def _prep_inputs(x, conv_state, weight, bias, conv_state_indices):
    x = np.asarray(x, dtype=np.float32)
    conv_state = np.asarray(conv_state, dtype=np.float32)
    weight = np.asarray(weight, dtype=np.float32)
    bias = np.asarray(bias, dtype=np.float32)
    idx = np.asarray(conv_state_indices)

    xs = np.empty((BATCH, T_XN, DIM), np.float32)
    xs[:, :ST, :] = conv_state[idx]
    xs[:, ST:, :] = x
    wb = _prep_wb(weight, bias)
    in_maps = [
        {"xs_s": xs[i * BPC:(i + 1) * BPC], "wb": wb}
        for i in range(NCORES)
    ]
    return in_maps, conv_state, idx


def kernel(x, conv_state, weight, bias, conv_state_indices):
    from concourse.bass_utils import run_bass_kernel_spmd

    in_maps, conv_state, idx = _prep_inputs(
        x, conv_state, weight, bias, conv_state_indices)

    nc = get_nc()
    res = run_bass_kernel_spmd(nc, in_maps, list(range(NCORES)), trace=TRACE)
    global LAST_RESULTS
    LAST_RESULTS = res
    out = np.concatenate([r["out_s"] for r in res.results], axis=0)

    updated = conv_state.copy()
    updated[idx] = np.asarray(x, dtype=np.float32)[:, SEQ - ST:, :]
    return out, updated


# revision 29
# speedup vs baseline: 14.7103x; 14.7103x over previous
"""Trainium2 Bass kernel for CausalConv1dUpdate (depthwise causal conv,
width=4, with state gather/scatter and SiLU).

Full-input contract: kernel(**inputs) takes the complete tensors, shards
the batch across 8 NeuronCores (data-parallel), runs the Bass kernel via
run_bass_kernel_spmd, and reassembles full outputs on the host.

Host prep: gathers conv_state rows by conv_state_indices and concatenates
[state, x] along time (one contiguous DMA source per core; the TT ISA
struct only carries one sync wait, so one input DMA per block is needed),
and pre-broadcasts weight+bias into the partition layout.

Device layout (per core, 32 sequences):
  - Two compute blocks of 16 sequences each.
  - SBUF partition dim = (c, j): c = d-chunk (8 of 512), j = seq-in-block
    (16) -> 128 partitions (p = c*16 + j); free dim = (t, u): t = time
    slot (19 = 3 state + 16 x), u = d-within-chunk. c-major packing keeps
    every DMA access pattern mergeable to <=3 dims.
  - conv: acc[t] = bias + sum_k w[k] * x_new[t+k]  (fp32, matching the
    reference accumulation order), then SiLU on the scalar engine.
  - new_state = x_new[:, -3:, :] = x[:, 13:16, :] is a pure slice of the
    input, so the host assembles updated_conv_state directly from x
    (no device traffic for it).
"""

import numpy as np

BATCH, SEQ, DIM, WIDTH, POOL = 256, 16, 4096, 4, 512
NCORES = 8
BPC = BATCH // NCORES       # sequences per core: 32
JBLK = 8                    # sequences per compute block
NBLK = BPC // JBLK          # 4
CCH = 16                    # d-chunks per partition group
U = DIM // CCH              # 256
T_XN = WIDTH - 1 + SEQ      # 19 time slots in x_new
ST = WIDTH - 1              # 3 state slots

_BUILT = None
# Offload the three tap-product multiplies to GPSIMD (vs all-DVE).
SPLIT = True
# Test-harness knobs (ignored in normal use): set TRACE=True before calling
# kernel() to profile the run; the BassKernelResults lands in LAST_RESULTS.
TRACE = False
LAST_RESULTS = None


def _build(act_name="Silu", reps=1, split=True):
    """Build the per-core Bass module.

    reps: unroll the whole pipeline N times (benchmarking variant — lets
    per-call wall-clock resolve the kernel time as a slope).
    split: offload the three tap-product multiplies to GPSIMD so they run
    concurrently with the DVE accumulation chain.
    """
    import concourse.mybir as mybir
    from concourse.bacc import Bacc
    from concourse.tile import TileContext

    f32 = mybir.dt.float32
    MUL = mybir.AluOpType.mult
    ADD = mybir.AluOpType.add
    SILU = getattr(mybir.ActivationFunctionType, act_name)

    # Bacc (not raw Bass): finalize() runs generate_event_semaphores(),
    # which legalizes sync info to <=1 wait per instruction — walrus
    # codegen on this toolchain rejects anything more.
    nc = Bacc()
    xs_s = nc.dram_tensor("xs_s", [BPC, T_XN, DIM], f32, kind="ExternalInput")
    # wb: weights+bias pre-broadcast on host to the partition layout:
    # wb[p, k, :] = weight[k, (p//JBLK)*U:(p//JBLK+1)*U] for k<WIDTH,
    # wb[p, WIDTH, :] = bias[(p//JBLK)*U:(p//JBLK+1)*U]
    wb = nc.dram_tensor("wb", [128, WIDTH + 1, U], f32, kind="ExternalInput")
    out_s = nc.dram_tensor("out_s", [BPC, SEQ, DIM], f32, kind="ExternalOutput")

    with TileContext(nc) as tc:
        with tc.tile_pool(name="wbp", bufs=1) as wbp, \
             tc.tile_pool(name="xnp", bufs=NBLK) as xnp, \
             tc.tile_pool(name="accp", bufs=NBLK) as accp, \
             tc.tile_pool(name="tmpp", bufs=1) as tmpp:
            wbt = wbp.tile([128, WIDTH + 1, U], f32)
            nc.sync.dma_start(out=wbt[:], in_=wb[:])
            # 1-element copies so DVE/GPSIMD observe the wb-DMA semaphore
            # here; every ISA struct carries at most one sync wait.
            warm = wbp.tile([128, 1], f32, tag="warm")
            nc.vector.tensor_copy(warm[:], wbt[:, 0, 0:1])
            if split:
                warm2 = wbp.tile([128, 1], f32, tag="warm2")
                nc.gpsimd.tensor_copy(warm2[:], wbt[:, 0, 0:1])

            mul_eng = nc.gpsimd if split else nc.vector

            def body(_iv=None):
                for b in range(NBLK):
                    lo, hi = b * JBLK, (b + 1) * JBLK
                    xn = xnp.tile([128, T_XN, U], f32, tag="xn", name="xn")
                    acc = accp.tile([128, SEQ, U], f32, tag="acc", name="acc")

                    xsrc = xs_s[lo:hi].rearrange("j t (c u) -> c (j t) u", u=U)
                    nc.sync.dma_start(out=xn[:], in_=xsrc)
                    if split:
                        # GPSIMD observes this block's input DMA before its
                        # tap products, so each of those carries at most the
                        # one WAR wait on DVE
                        pscr = wbp.tile([128, 1], f32, tag="pscr", name="pscr")
                        nc.gpsimd.tensor_copy(pscr[:], xn[:, 0, 0:1])

                    # DVE: tap-0 product, +bias, and the accumulation adds;
                    # GPSIMD (if split): the three other tap products into
                    # two alternating tmp slots.
                    w0 = wbt[:, 0:1, :].broadcast_to([128, SEQ, U])
                    nc.vector.tensor_tensor(acc[:], xn[:, 0:SEQ, :], w0, MUL)
                    bias_b = wbt[:, WIDTH:WIDTH + 1, :].broadcast_to([128, SEQ, U])
                    nc.vector.tensor_tensor(acc[:], acc[:], bias_b, ADD)
                    tmps = []
                    for k in range(1, WIDTH):
                        tg = f"tmp{k % 2}"
                        tmp = tmpp.tile([128, SEQ, U], f32, tag=tg, name=tg)
                        wk = wbt[:, k:k + 1, :].broadcast_to([128, SEQ, U])
                        mul_eng.tensor_tensor(tmp[:], xn[:, k:k + SEQ, :], wk, MUL)
                        tmps.append(tmp)
                        # interleave the add as soon as each product lands
                    for tmp in tmps:
                        nc.vector.tensor_tensor(acc[:], acc[:], tmp[:], ADD)

                    accf = acc[:].rearrange("p t u -> p (t u)")
                    nc.scalar.activation(accf, accf, SILU)

                    odst = out_s[lo:hi].rearrange("j t (c u) -> c (j t) u", u=U)
                    # issue from the scalar engine: program order after the
                    # in-place SiLU makes the RAW dep free (DMA ISA structs
                    # carry only one sync wait)
                    nc.scalar.dma_start(out=odst, in_=acc[:])

            if reps == 1:
                body()
            else:
                # benchmark variant: the For_i back-edge barrier + sem reset
                # legalizes all cross-iteration hazards
                with tc.For_i(0, reps, 1) as _i:
                    body(_i)

    nc.finalize()
    return nc


def get_nc(act_name="Silu"):
    global _BUILT
    if _BUILT is None:
        _BUILT = _build(act_name, split=SPLIT)
    return _BUILT


def _prep_wb(weight, bias):
    wb = np.empty((128, WIDTH + 1, U), np.float32)
    c = np.arange(128) // JBLK
    wr = weight.reshape(WIDTH, CCH, U)
    for k in range(WIDTH):
        wb[:, k, :] = wr[k][c]
    wb[:, WIDTH, :] = bias.reshape(CCH, U)[c]
    return wb


def _prep_inputs(x, conv_state, weight, bias, conv_state_indices):
    x = np.asarray(x, dtype=np.float32)
    conv_state = np.asarray(conv_state, dtype=np.float32)
    weight = np.asarray(weight, dtype=np.float32)
    bias = np.asarray(bias, dtype=np.float32)
    idx = np.asarray(conv_state_indices)

    xs = np.empty((BATCH, T_XN, DIM), np.float32)
    xs[:, :ST, :] = conv_state[idx]
    xs[:, ST:, :] = x
    wb = _prep_wb(weight, bias)
    in_maps = [
        {"xs_s": xs[i * BPC:(i + 1) * BPC], "wb": wb}
        for i in range(NCORES)
    ]
    return in_maps, conv_state, idx


def kernel(x, conv_state, weight, bias, conv_state_indices):
    from concourse.bass_utils import run_bass_kernel_spmd

    in_maps, conv_state, idx = _prep_inputs(
        x, conv_state, weight, bias, conv_state_indices)

    nc = get_nc()
    res = run_bass_kernel_spmd(nc, in_maps, list(range(NCORES)), trace=TRACE)
    global LAST_RESULTS
    LAST_RESULTS = res
    out = np.concatenate([r["out_s"] for r in res.results], axis=0)

    updated = conv_state.copy()
    updated[idx] = np.asarray(x, dtype=np.float32)[:, SEQ - ST:, :]
    return out, updated
